# revision 108
# baseline (speedup 1.0000x reference)
"""Trainium2 Bass kernel for the capsule-routing layer (nn_Caps_Layer), v3b.

Computation (per batch b of x [B, S, D], W [D, 25]):
  u_hat = (x_b @ W).reshape(S, 5, 5)           # [S, n, k]
  b0 = 0;  for 4 routing iters:
    c = softmax_n(b)
    v[n,k] = sum_s c[n,s] u_hat[s,n,k]
    out = v / sqrt(sum_k v^2 + 1e-7)
    b[n,s] = sum_k out[n,k] u_hat[s,n,k]
Returns out [B, 5, 5].

Sharding: pure data-parallel over batch across 8 NeuronCores (16 batches
each); W replicated; no collectives.

Design notes (from perfetto traces of prior versions):
  - x is cast to fp16 AND transposed on the host as part of shard prep:
    DRAM layout per batch is [d_lo(128 part), db(6), s(512)] so the
    device never transposes x.  Halves HBM traffic (memory-regime floor
    ~38us/core) and removes 384 PE transposes + psum copies per core.
  - v3b: xt pool bufs=16 — with bufs=6 the x DMA issue for batch b+6
    carried a buffer-reuse wait on batch b's matmuls, which serialized
    the DMA wire behind compute (x was still arriving at 82us of an
    84us span).  16 buffers fit easily in SBUF (96KB of 208KB per
    partition) and let both HWDGE queues stream the wire flat out.
  - u_hatT accumulates in PSUM from 6 fp16 matmuls per batch
    (lhsT = W block [128d, 32(zero-padded)], rhs = xT block [128d, 512]).
    4 batches stack at 32-partition offsets in one [128, 512] PSUM bank
    (PE psum outputs must be 32-aligned: explicit tile_position); the
    psum->sbuf copy and transpose-back amortize 4x, with a gather matrix
    (identity columns at 32j+nk) compacting the stacking on the way back.
  - routing in natural layout [128 s_lo, (sh, b, n, k)]:
      * per-GROUP tile-pool tags so different groups' chains overlap
        (shared tags had bufs=2 rotation serializing group g+1 on g),
      * iter-0 shortcut: c uniform -> v0 = sum_s u_hat via one DVE
        reduce over u_hatT + a tiny gather-transpose,
      * squash via rnrm = exp(-0.5*ln(cs^2*|v|^2+eps)) on ACT: Ln/Exp/
        Square share one activation table set => no ACT table reloads
        (Sqrt<->Exp alternation costs 1.3us per switch); iter-0's cs=1/5
        folds into the Exp bias (ln(1/5)),
      * rnrm folds into vr = v*rnrm,
      * v-sum via 4 accumulating ones-matmuls over the sh blocks with an
        immediate tiny copy off psum so the shared pv buffer frees fast,
      * f32 routing tiles except the PE-bound t = c*u_hat product
        (fp16 keeps the ones-matmul at 1 cyc/row),
  - HAM clock gate: PE idles between DMA-bound matmul bursts drop the
    clock to 4/8 (fp16 512-row matmul: 634ns vs 215ns) — tiny warm
    matmuls after each batch + at iteration points keep activity up.
  - x DMAs alternate between the two HWDGE queues (sync/scalar).
"""

from contextlib import ExitStack

import numpy as np
import ml_dtypes

import concourse.bass as bass
import concourse.tile as tile
from concourse import mybir, masks

F32 = mybir.dt.float32
FP16 = mybir.dt.float16
AX = mybir.AxisListType
OP = mybir.AluOpType
AF = mybir.ActivationFunctionType

N_CORES = 8
B_FULL, S, D = 128, 512, 768
NCAP, KDIM = 5, 5
NK = NCAP * KDIM  # 25
ROUTINGS = 4
T_EPS = 1e-7
LN_CS = float(np.log(1.0 / NCAP))  # iter-0 softmax is uniform: c = 1/5

ND = D // 128   # 6 d-blocks
NSB = S // 128  # 4 s-blocks (= sh)
QUAD = 4        # max batches stacked per PSUM u_hatT tile


def emit(ctx, tc, out, x, w, b_loc=16, groups=(4, 4, 4, 4)):
    """Emit the per-core kernel IR.

    out: [1, b_loc*25] f32; x: [b_loc*128, ND*S] fp16 host-transposed so
    row (b*128 + d_lo) holds [db, s]; w: [128, ND*32] fp16 host-arranged
    so row d_lo holds [db, nk(zero-padded to 32)].
    """
    nc = tc.nc
    groups = list(groups)
    assert sum(groups) == b_loc
    assert all(1 <= gg <= 2 * QUAD for gg in groups)

    const_pool = ctx.enter_context(tc.tile_pool(name="const", bufs=1))
    xt_pool = ctx.enter_context(tc.tile_pool(name="xt", bufs=8))
    pu_pool = ctx.enter_context(tc.tile_pool(name="pu", bufs=1, space="PSUM"))
    uhT_pool = ctx.enter_context(tc.tile_pool(name="uhT", bufs=2))
    pnat_pool = ctx.enter_context(tc.tile_pool(name="pnat", bufs=1, space="PSUM"))
    uh_pool = ctx.enter_context(tc.tile_pool(name="uh", bufs=1))
    rt_pool = ctx.enter_context(tc.tile_pool(name="rt", bufs=1))
    pv_pool = ctx.enter_context(tc.tile_pool(name="pv", bufs=2, space="PSUM"))
    pv0_pool = ctx.enter_context(tc.tile_pool(name="pv0", bufs=1, space="PSUM"))
    pvb_pool = ctx.enter_context(tc.tile_pool(name="pvb", bufs=2, space="PSUM"))

    # --- constants ---
    ident = const_pool.tile([128, 128], F32)
    masks.make_identity(nc, ident[:])
    ident_b = const_pool.tile([128, 128], FP16)
    nc.scalar.copy(ident_b[:], ident[:])
    # gather matrix: identity columns at 32j+nk — used as transpose rhs
    # to compact the 32-aligned psum stacking down to (j, nk).
    gat_f = const_pool.tile([128, QUAD * NK], F32)
    gat_h = const_pool.tile([128, QUAD * NK], FP16)
    for j in range(QUAD):
        nc.scalar.copy(
            gat_f[:, j * NK:(j + 1) * NK], ident[:, j * 32:j * 32 + NK]
        )
        nc.scalar.copy(
            gat_h[:, j * NK:(j + 1) * NK], ident[:, j * 32:j * 32 + NK]
        )
    w_sb = const_pool.tile([128, ND * 32], FP16)
    nc.scalar.dma_start(w_sb[:], w[:, :])
    # mask for the b-update transpose-matmul: mask[32j+5n+k, 5j+n] = 1.
    # M = mask * vr_col then blog_nat = uhT^T @ M per sh block — the
    # whole b-update (was tmp+blog, 1.1us of DVE per iter) moves to PE.
    # Built as a grouped reduce over identity columns: mask col (j,n) =
    # sum_k ident col (32j+5n+k).
    mask = const_pool.tile([128, QUAD * NCAP], F32)
    nc.vector.reduce_sum(
        mask[:],
        ident[:].rearrange("p (j x) -> p j x", j=QUAD)[:, :, 0:NK]
        .rearrange("p j (n k) -> p j n k", n=NCAP),
        axis=AX.X,
    )
    ones_col_b = const_pool.tile([128, 1], FP16)
    nc.gpsimd.memset(ones_col_b[:], 1.0)
    ones_row_b = const_pool.tile([1, 128], FP16)
    nc.gpsimd.memset(ones_row_b[:], 1.0)
    eps1 = const_pool.tile([1, 1], F32)
    nc.gpsimd.memset(eps1[:], T_EPS)
    ln_cs = const_pool.tile([1, 1], F32)
    nc.gpsimd.memset(ln_cs[:], LN_CS)

    def warm_pe(n=1):
        """Tiny REGULAR matmuls keep the HAM clock gate at 8/8."""
        for _ in range(n):
            wps = pv_pool.tile([1, 512], F32, tag="pv")
            nc.tensor.matmul(
                wps[0:1, 0:128], ones_col_b[:], ident_b[:], start=True, stop=True
            )

    # HAM warm-up overlapping the first DMA.
    warm_pe(24)

    # ---------------- per-quad phase 1 ----------------
    qcount = [0]

    xts = {}

    def issue_x(b):
        """Allocate the batch's SBUF tile and enqueue its DMA."""
        xt = xt_pool.tile([128, ND * S], FP16, tag="xt", name=f"xt{b}")
        eng = nc.sync if b % 2 == 0 else nc.scalar
        eng.dma_start(xt[:], x[b * 128:(b + 1) * 128, :])
        xts[b] = xt

    pus = {}

    def mm_batch(b):
        """6 u_hatT matmuls for one batch into its quad's PSUM tile."""
        g, j = b // QUAD, b % QUAD
        if j == 0:
            pus[g] = pu_pool.tile([128, S], F32, tag="pu", name=f"pu{g}")
        pu = pus[g]
        if b not in xts:
            issue_x(b)
        xt = xts[b]
        for db in range(ND):
            nc.tensor.matmul(
                pu[j * 32:(j + 1) * 32, :],
                w_sb[:, db * 32:(db + 1) * 32],
                xt[:, db * S:(db + 1) * S],
                start=(db == 0),
                stop=(db == ND - 1),
                tile_position=(0, j * 32),
            )
        warm_pe()

    def phase1_post(g):
        """psum->sbuf cast + iter-0 sum + transpose-back for one quad."""
        pu = pus.pop(g)
        # psum->sbuf fp16 cast on ACT (idle in phase 1; keeps DVE free).
        # Per-group tag: uhT persists through routing as the b-update
        # matmul source.
        uhT = uhT_pool.tile([128, S], FP16, tag=f"uhT{g}", name=f"uhT{g}")
        nc.scalar.copy(uhT[:], pu[:])
        # iter-0 shortcut input: v0T[32j+nk, 1] = sum_s u_hatT
        v0T = uhT_pool.tile([128, 1], F32, tag="v0T")
        nc.vector.reduce_sum(v0T[:], uhT[:], axis=AX.X)
        # transpose back + compact: pnat [128 s_lo, (sh, j, nk)] fp16
        pnat = pnat_pool.tile([128, NSB * QUAD * NK], FP16, tag="pnat")
        for sh in range(NSB):
            nc.tensor.matmul(
                pnat[:, sh * QUAD * NK:(sh + 1) * QUAD * NK],
                uhT[:, sh * 128:(sh + 1) * 128],
                gat_h[:, 0:QUAD * NK],
                is_transpose=True,
                start=True,
                stop=True,
            )
        return pnat, v0T, uhT

    # ---------------- routing: one iteration of one group ----------------
    def route_iter(st, it):
        """Emit routing iteration `it` for group state `st`.

        Engines run their instruction queues IN ORDER, so a full group
        chain emitted contiguously blocks every later group's ready work
        behind its semaphore waits.  The main loop therefore calls this
        in a diagonal wavefront over (group, iter) so each engine queue
        interleaves independent groups' ops.
        """
        g, b_off, G = st["g"], st["b_off"], st["G"]
        uh_ap = st["uh_ap"]
        blog = st["blog"]
        v_src = st["v_src"]
        if True:
            cs = 1.0 / NCAP if it == 0 else 1.0
            t_src = st["uh"]
            if it > 0:
                # c = softmax_n(blog); t = c * u_hat  (blog lives in PSUM
                # f32, written by the b-update transpose-matmuls)
                expb = rt_pool.tile([128, NSB * G * NCAP], FP16, tag=f"expb{g}")
                nc.scalar.activation(expb[:], blog[:], AF.Exp)
                den = rt_pool.tile([128, NSB * G], F32, tag=f"den{g}")
                nc.vector.reduce_sum(
                    den[:],
                    expb[:].rearrange("p (sb n) -> p sb n", n=NCAP),
                    axis=AX.X,
                )
                rden = rt_pool.tile([128, NSB * G], F32, tag=f"rden{g}")
                nc.vector.reciprocal(rden[:], den[:])
                # flat 3D APs keep the DVE off its slow generic-AP path
                # (measured: the 5D broadcast forms ran ~40% slower)
                c = rt_pool.tile([128, NSB * G * NCAP], FP16, tag=f"c{g}")
                nc.vector.tensor_tensor(
                    c[:].rearrange("p (sb n) -> p sb n", n=NCAP),
                    expb[:].rearrange("p (sb n) -> p sb n", n=NCAP),
                    rden[:].unsqueeze(2).broadcast_to((128, NSB * G, NCAP)),
                    op=OP.mult,
                )
                t = rt_pool.tile([128, NSB * G * NK], FP16, tag=f"t{g}")
                nc.vector.tensor_tensor(
                    t[:].rearrange("p (sbn k) -> p sbn k", k=KDIM),
                    st["uh"][:].rearrange("p (sbn k) -> p sbn k", k=KDIM),
                    c[:].unsqueeze(2)
                    .broadcast_to((128, NSB * G * NCAP, KDIM)),
                    op=OP.mult,
                )
                # v[b,n,k] = sum_s t: 4 accumulating ones-matmuls over
                # the sh blocks.  v stays in PSUM — squash (ACT) and
                # vr/outs (DVE, one-psum-input rule satisfied) read it
                # there directly, so no copy-off and one fewer chain hop.
                pv = pv_pool.tile([1, 512], F32, tag="pv")
                t_ap = t[:].rearrange("p (sh rest) -> p sh rest", sh=NSB)
                for sh in range(NSB):
                    nc.tensor.matmul(
                        pv[0:1, 0:G * NK],
                        ones_col_b[:],
                        t_ap[:, sh],
                        start=(sh == 0),
                        stop=(sh == NSB - 1),
                    )
                v_src = pv
                st["v_src"] = pv
            # ---- squash side-chain: rnrm = exp(-0.5*ln(cs^2*|v|^2+eps))
            # Square/Ln/Exp all live in one ACT table set (no reloads).
            v_ap = v_src[0:1, 0:G * NK]
            sq = rt_pool.tile([1, G * NK], F32, tag=f"sq{g}")
            nc.scalar.activation(sq[:], v_ap, AF.Square)
            s2 = rt_pool.tile([1, G * NCAP], F32, tag=f"s2{g}")
            nc.vector.reduce_sum(
                s2[:], sq[:].rearrange("p (bn k) -> p bn k", k=KDIM), axis=AX.X
            )
            lnv = rt_pool.tile([1, G * NCAP], F32, tag=f"lnv{g}")
            nc.scalar.activation(lnv[:], s2[:], AF.Ln, bias=eps1[:], scale=cs * cs)
            rnrm = rt_pool.tile([1, G * NCAP], F32, tag=f"rnrm{g}")
            nc.scalar.activation(
                rnrm[:], lnv[:], AF.Exp,
                bias=ln_cs[:] if it == 0 else 0.0, scale=-0.5,
            )
            rnrm_b = rnrm[:].unsqueeze(2).broadcast_to((1, G * NCAP, KDIM))
            if it < ROUTINGS - 1:
                # vr = v * rnrm (iter-0 cs folded into rnrm via Exp
                # bias), written straight into 32-aligned blocks of a
                # [1,128] row (pads zeroed once per group).
                # b-update on the PE (was tmp+blog: 1.1us of DVE/iter):
                # vr row -> column via ones-matmul, M = mask*vr_col,
                # then blog_nat[s,(sh,b,n)] = sum_p uhT[p,s]*M[p,(b,n)]
                # per sh block in NORMAL matmul mode (general rhs).
                vr32 = rt_pool.tile([1, 128], FP16, tag=f"vr32{g}")
                if it == 0:
                    nc.gpsimd.memset(vr32[:], 0.0)
                nc.vector.tensor_tensor(
                    vr32[:].rearrange("p (j x) -> p j x", j=QUAD)
                    [:, :, 0:NK].rearrange("p j (n k) -> p j n k", n=NCAP),
                    v_ap.rearrange("p (j n k) -> p j n k", j=G, n=NCAP),
                    rnrm[:].rearrange("p (j n) -> p j n", j=G)
                    .unsqueeze(3).broadcast_to((1, G, NCAP, KDIM)),
                    op=OP.mult,
                )
                pvc = pvb_pool.tile([128, 1], F32, tag="pvc", bufs=1)
                nc.tensor.matmul(
                    pvc[:, 0:1], vr32[:], ones_col_b[0:1, 0:1],
                    start=True, stop=True,
                )
                em = rt_pool.tile([128, G * NCAP], FP16, tag=f"em{g}")
                nc.vector.tensor_tensor(
                    em[:],
                    mask[:, 0:G * NCAP],
                    pvc[:, 0:1].broadcast_to((128, G * NCAP)),
                    op=OP.mult,
                )
                blog = pvb_pool.tile(
                    [128, NSB * G * NCAP], F32, tag=f"pbl{g % 2}", bufs=1
                )
                for sh in range(NSB):
                    nc.tensor.matmul(
                        blog[:, sh * G * NCAP:(sh + 1) * G * NCAP],
                        st["uhT"][:, sh * 128:(sh + 1) * 128],
                        em[:],
                        start=True,
                        stop=True,
                    )
                st["blog"] = blog
            else:
                outs = rt_pool.tile([1, G * NK], F32, tag=f"outs{g}")
                nc.vector.tensor_tensor(
                    outs[:].rearrange("p (bn k) -> p bn k", k=KDIM),
                    v_ap.rearrange("p (bn k) -> p bn k", k=KDIM),
                    rnrm_b,
                    op=OP.mult,
                )
                nc.sync.dma_start(
                    out[0:1, b_off * NK:(b_off + G) * NK], outs[0:1, :]
                )

    # ---------------- main loop: diagonal wavefront ----------------
    # Slot s emits phase 1 of group s (one quad) plus routing iteration
    # (s - 1 - g) of every group g that has data — so every engine's
    # queue interleaves independent groups' ops.
    ngr = len(groups)
    states = []

    def phase1_group(g, G, b_off):
        assert G == QUAD
        uh = uh_pool.tile([128, NSB * G * NK], FP16, tag=f"uh{g}")
        pv0 = pv0_pool.tile([1, G * NK], F32, tag="pv0")
        pnat, v0T, uhT = phase1_post(g)
        # groups == quads, so pnat IS uh's layout: one plain ACT copy
        nc.scalar.copy(uh[:], pnat[:])
        # v0 row: pv0[0, (b, nk)] <- gather-transpose of v0T
        nc.tensor.matmul(
            pv0[0:1, 0:G * NK],
            v0T[:],
            gat_f[:, 0:G * NK],
            is_transpose=True,
            start=True,
            stop=True,
        )
        return {
            "g": g, "G": G, "b_off": b_off,
            "uh": uh, "uhT": uhT,
            "uh_ap": uh[:].rearrange(
                "p (sh b n k) -> p sh b n k", sh=NSB, b=G, n=NCAP
            ),
            "blog": None, "v_src": pv0,
        }

    # Readiness-interleaved schedule.  Groups 0-1 (first 8 batches:
    # exactly the 4 wait-free DMA issues each HWDGE queue allows) load
    # and route while the wire streams quads 2-3.  The second half's
    # DMA issues (which carry 4-deep semaphore-rotation waits) AND
    # quads 2-3's per-batch matmuls/posts are threaded BETWEEN routing
    # iterations at the points their inputs become ready — on in-order
    # engine queues nothing ever waits ahead of ready work, and groups
    # 2-3's routing starts as soon as their data lands instead of
    # queueing behind the whole first-half routing block.
    assert groups == [QUAD] * 4 and b_loc == 16
    for b in range(8):
        mm_batch(b)
        if b == 3:
            states.append(phase1_group(0, QUAD, 0))
    # R(0,0) before phase1_group(1): pv0 is a single rotating bank, so
    # each group's gather-transpose write must be emitted after the
    # previous group's iter-0 readers.
    route_iter(states[0], 0)
    states.append(phase1_group(1, QUAD, QUAD))
    issue_x(8)
    route_iter(states[1], 0)
    issue_x(9)
    route_iter(states[0], 1)
    issue_x(10)
    issue_x(11)
    mm_batch(8)
    mm_batch(9)
    route_iter(states[1], 1)
    mm_batch(10)
    mm_batch(11)
    route_iter(states[0], 2)
    issue_x(12)
    issue_x(13)
    states.append(phase1_group(2, QUAD, 8))
    route_iter(states[1], 2)
    mm_batch(12)
    mm_batch(13)
    route_iter(states[2], 0)
    route_iter(states[0], 3)
    issue_x(14)
    issue_x(15)
    mm_batch(14)
    mm_batch(15)
    route_iter(states[1], 3)
    states.append(phase1_group(3, QUAD, 12))
    # group 3's chain is the tail: start it before R(2,1) so the final
    # pair pipelines from the earliest possible point
    route_iter(states[3], 0)
    route_iter(states[2], 1)
    route_iter(states[2], 2)
    route_iter(states[3], 1)
    route_iter(states[2], 3)
    route_iter(states[3], 2)
    route_iter(states[3], 3)


def legalize_waits(nc):
    """This toolchain's walrus codegen accepts at most ONE sync wait per
    instruction ("Too many sync wait commands" otherwise) — and PE Matmult
    appears to take none safely. Hoist excess waits onto wait-only
    EventSemaphore instructions inserted just before, on the same engine
    (same pattern walrus already accepts for Tile's engine barriers)."""
    n = 0
    for fn in nc.m.functions:
        for blk in fn.blocks:
            new = []
            for inst in blk.instructions:
                si = inst.sync_info
                if si is not None and len(si.on_wait) > 0:
                    waits = list(si.on_wait)
                    keep = 0 if type(inst).__name__ == "InstMatmult" else 1
                    if len(waits) > keep:
                        for wt in waits[: len(waits) - keep]:
                            ev = mybir.InstEventSemaphore(
                                name=f"I-waitfix-{nc.next_id()}"
                            )
                            ev.engine = inst.engine
                            ev.sync_info = mybir.SyncInfo(on_wait=[wt], on_update=[])
                            new.append(ev)
                            n += 1
                        si.on_wait = waits[len(waits) - keep:]
                new.append(inst)
            blk.instructions = new
    return n


def build_caps_kernel(b_loc=16, groups=(4, 4, 4, 4)):
    nc = bass.Bass(trn_type="TRN2", debug=False, target_bir_lowering=False)
    x = nc.dram_tensor("x", [b_loc * 128, ND * S], FP16, kind="ExternalInput").ap()
    w = nc.dram_tensor("w", [128, ND * 32], FP16, kind="ExternalInput").ap()
    out = nc.dram_tensor("out", [1, b_loc * NK], F32, kind="ExternalOutput").ap()
    with tile.TileContext(nc) as tc:
        with ExitStack() as ctx:
            emit(ctx, tc, out, x, w, b_loc=b_loc, groups=groups)
    legalize_waits(nc)
    return nc


_KERNEL_CFG = dict(groups=(4, 4, 4, 4))


def _prep_inputs(x: np.ndarray, W: np.ndarray, b_loc: int):
    """Shard + host-side layout prep: fp16 cast and d-major transpose."""
    xb = x.astype(np.float16)
    wb = W.astype(np.float16)
    # W [D, 25] -> [d_lo(128), db(6)*32] with cols 25..31 zero-padded
    w_dev = np.zeros((128, ND, 32), dtype=np.float16)
    w_dev[:, :, :NK] = wb.reshape(ND, 128, NK).transpose(1, 0, 2)
    w_dev = np.ascontiguousarray(w_dev.reshape(128, ND * 32))
    in_maps = []
    for i in range(N_CORES):
        shard = xb[i * b_loc:(i + 1) * b_loc]  # [b_loc, S, D]
        # [b, s, db, d_lo] -> [b, d_lo, db, s]
        xt = np.ascontiguousarray(
            shard.reshape(b_loc, S, ND, 128).transpose(0, 3, 2, 1)
        ).reshape(b_loc * 128, ND * S)
        in_maps.append({"x": xt, "w": w_dev})
    return in_maps


def kernel(x: np.ndarray, W: np.ndarray) -> np.ndarray:
    from concourse.bass_utils import run_bass_kernel_spmd

    B, S_, D_ = x.shape
    assert (B, S_, D_) == (B_FULL, S, D)
    b_loc = B // N_CORES
    nc = build_caps_kernel(b_loc=b_loc, **_KERNEL_CFG)
    in_maps = _prep_inputs(x, W, b_loc)
    res = run_bass_kernel_spmd(nc, in_maps, core_ids=list(range(N_CORES)))
    outs = [res.results[i]["out"].reshape(b_loc, NCAP, KDIM) for i in range(N_CORES)]
    return np.concatenate(outs, axis=0).astype(np.float32)


# revision 109
# speedup vs baseline: 1.0337x; 1.0337x over previous
"""Trainium2 Bass kernel for the capsule-routing layer (nn_Caps_Layer), v3b.

Computation (per batch b of x [B, S, D], W [D, 25]):
  u_hat = (x_b @ W).reshape(S, 5, 5)           # [S, n, k]
  b0 = 0;  for 4 routing iters:
    c = softmax_n(b)
    v[n,k] = sum_s c[n,s] u_hat[s,n,k]
    out = v / sqrt(sum_k v^2 + 1e-7)
    b[n,s] = sum_k out[n,k] u_hat[s,n,k]
Returns out [B, 5, 5].

Sharding: pure data-parallel over batch across 8 NeuronCores (16 batches
each); W replicated; no collectives.

Design notes (from perfetto traces of prior versions):
  - x is cast to fp16 AND transposed on the host as part of shard prep:
    DRAM layout per batch is [d_lo(128 part), db(6), s(512)] so the
    device never transposes x.  Halves HBM traffic (memory-regime floor
    ~38us/core) and removes 384 PE transposes + psum copies per core.
  - v3b: xt pool bufs=16 — with bufs=6 the x DMA issue for batch b+6
    carried a buffer-reuse wait on batch b's matmuls, which serialized
    the DMA wire behind compute (x was still arriving at 82us of an
    84us span).  16 buffers fit easily in SBUF (96KB of 208KB per
    partition) and let both HWDGE queues stream the wire flat out.
  - u_hatT accumulates in PSUM from 6 fp16 matmuls per batch
    (lhsT = W block [128d, 32(zero-padded)], rhs = xT block [128d, 512]).
    4 batches stack at 32-partition offsets in one [128, 512] PSUM bank
    (PE psum outputs must be 32-aligned: explicit tile_position); the
    psum->sbuf copy and transpose-back amortize 4x, with a gather matrix
    (identity columns at 32j+nk) compacting the stacking on the way back.
  - routing in natural layout [128 s_lo, (sh, b, n, k)]:
      * per-GROUP tile-pool tags so different groups' chains overlap
        (shared tags had bufs=2 rotation serializing group g+1 on g),
      * iter-0 shortcut: c uniform -> v0 = sum_s u_hat via one DVE
        reduce over u_hatT + a tiny gather-transpose,
      * squash via rnrm = exp(-0.5*ln(cs^2*|v|^2+eps)) on ACT: Ln/Exp/
        Square share one activation table set => no ACT table reloads
        (Sqrt<->Exp alternation costs 1.3us per switch); iter-0's cs=1/5
        folds into the Exp bias (ln(1/5)),
      * rnrm folds into vr = v*rnrm,
      * v-sum via 4 accumulating ones-matmuls over the sh blocks with an
        immediate tiny copy off psum so the shared pv buffer frees fast,
      * f32 routing tiles except the PE-bound t = c*u_hat product
        (fp16 keeps the ones-matmul at 1 cyc/row),
  - HAM clock gate: PE idles between DMA-bound matmul bursts drop the
    clock to 4/8 (fp16 512-row matmul: 634ns vs 215ns) — tiny warm
    matmuls after each batch + at iteration points keep activity up.
  - x DMAs alternate between the two HWDGE queues (sync/scalar).
"""

from contextlib import ExitStack

import numpy as np
import ml_dtypes

import concourse.bass as bass
import concourse.tile as tile
from concourse import mybir, masks

F32 = mybir.dt.float32
FP16 = mybir.dt.float16
AX = mybir.AxisListType
OP = mybir.AluOpType
AF = mybir.ActivationFunctionType

N_CORES = 8
B_FULL, S, D = 128, 512, 768
NCAP, KDIM = 5, 5
NK = NCAP * KDIM  # 25
ROUTINGS = 4
T_EPS = 1e-7
LN_CS = float(np.log(1.0 / NCAP))  # iter-0 softmax is uniform: c = 1/5

ND = D // 128   # 6 d-blocks
NSB = S // 128  # 4 s-blocks (= sh)
QUAD = 4        # max batches stacked per PSUM u_hatT tile


def emit(ctx, tc, out, x, w, b_loc=16, groups=(4, 4, 4, 4)):
    """Emit the per-core kernel IR.

    out: [1, b_loc*25] f32; x: [b_loc*128, ND*S] fp16 host-transposed so
    row (b*128 + d_lo) holds [db, s]; w: [128, ND*32] fp16 host-arranged
    so row d_lo holds [db, nk(zero-padded to 32)].
    """
    nc = tc.nc
    groups = list(groups)
    assert sum(groups) == b_loc
    assert all(1 <= gg <= 2 * QUAD for gg in groups)

    const_pool = ctx.enter_context(tc.tile_pool(name="const", bufs=1))
    xt_pool = ctx.enter_context(tc.tile_pool(name="xt", bufs=6))
    pu_pool = ctx.enter_context(tc.tile_pool(name="pu", bufs=1, space="PSUM"))
    uhT_pool = ctx.enter_context(tc.tile_pool(name="uhT", bufs=2))
    pnat_pool = ctx.enter_context(tc.tile_pool(name="pnat", bufs=1, space="PSUM"))
    uh_pool = ctx.enter_context(tc.tile_pool(name="uh", bufs=1))
    rt_pool = ctx.enter_context(tc.tile_pool(name="rt", bufs=1))
    pv_pool = ctx.enter_context(tc.tile_pool(name="pv", bufs=2, space="PSUM"))
    pv0_pool = ctx.enter_context(tc.tile_pool(name="pv0", bufs=1, space="PSUM"))
    pvb_pool = ctx.enter_context(tc.tile_pool(name="pvb", bufs=2, space="PSUM"))

    # --- constants ---
    ident = const_pool.tile([128, 128], F32)
    masks.make_identity(nc, ident[:])
    ident_b = const_pool.tile([128, 128], FP16)
    nc.scalar.copy(ident_b[:], ident[:])
    # gather matrix: identity columns at 32j+nk — used as transpose rhs
    # to compact the 32-aligned psum stacking down to (j, nk).
    gat_f = const_pool.tile([128, QUAD * NK], F32)
    gat_h = const_pool.tile([128, QUAD * NK], FP16)
    for j in range(QUAD):
        nc.scalar.copy(
            gat_f[:, j * NK:(j + 1) * NK], ident[:, j * 32:j * 32 + NK]
        )
        nc.scalar.copy(
            gat_h[:, j * NK:(j + 1) * NK], ident[:, j * 32:j * 32 + NK]
        )
    w_sb = const_pool.tile([128, ND * 32], FP16)
    nc.scalar.dma_start(w_sb[:], w[:, :])
    # mask for the b-update transpose-matmul: mask[32j+5n+k, 5j+n] = 1.
    # M = mask * vr_col then blog_nat = uhT^T @ M per sh block — the
    # whole b-update (was tmp+blog, 1.1us of DVE per iter) moves to PE.
    # Built as a grouped reduce over identity columns: mask col (j,n) =
    # sum_k ident col (32j+5n+k).
    mask = const_pool.tile([128, QUAD * NCAP], F32)
    nc.vector.reduce_sum(
        mask[:],
        ident[:].rearrange("p (j x) -> p j x", j=QUAD)[:, :, 0:NK]
        .rearrange("p j (n k) -> p j n k", n=NCAP),
        axis=AX.X,
    )
    ones_col_b = const_pool.tile([128, 1], FP16)
    nc.gpsimd.memset(ones_col_b[:], 1.0)
    ones_row_b = const_pool.tile([1, 128], FP16)
    nc.gpsimd.memset(ones_row_b[:], 1.0)
    eps1 = const_pool.tile([1, 1], F32)
    nc.gpsimd.memset(eps1[:], T_EPS)
    ln_cs = const_pool.tile([1, 1], F32)
    nc.gpsimd.memset(ln_cs[:], LN_CS)

    def warm_pe(n=1):
        """Tiny REGULAR matmuls keep the HAM clock gate at 8/8."""
        for _ in range(n):
            wps = pv_pool.tile([1, 512], F32, tag="pv")
            nc.tensor.matmul(
                wps[0:1, 0:128], ones_col_b[:], ident_b[:], start=True, stop=True
            )

    # HAM warm-up overlapping the first DMA.
    warm_pe(24)

    # ---------------- per-quad phase 1 ----------------
    qcount = [0]

    xts = {}

    def issue_x(b):
        """Allocate the batch's SBUF tile and enqueue its DMA."""
        xt = xt_pool.tile([128, ND * S], FP16, tag="xt", name=f"xt{b}")
        eng = nc.sync if b % 2 == 0 else nc.scalar
        eng.dma_start(xt[:], x[b * 128:(b + 1) * 128, :])
        xts[b] = xt

    pus = {}

    def mm_batch(b):
        """6 u_hatT matmuls for one batch into its quad's PSUM tile."""
        g, j = b // QUAD, b % QUAD
        if j == 0:
            pus[g] = pu_pool.tile([128, S], F32, tag="pu", name=f"pu{g}")
        pu = pus[g]
        if b not in xts:
            issue_x(b)
        xt = xts[b]
        for db in range(ND):
            nc.tensor.matmul(
                pu[j * 32:(j + 1) * 32, :],
                w_sb[:, db * 32:(db + 1) * 32],
                xt[:, db * S:(db + 1) * S],
                start=(db == 0),
                stop=(db == ND - 1),
                tile_position=(0, j * 32),
            )
        warm_pe()

    def phase1_post(g):
        """psum->sbuf cast + iter-0 sum + transpose-back for one quad."""
        pu = pus.pop(g)
        # psum->sbuf fp16 cast on ACT (idle in phase 1; keeps DVE free).
        # Per-group tag: uhT persists through routing as the b-update
        # matmul source.
        uhT = uhT_pool.tile([128, S], FP16, tag=f"uhT{g}", name=f"uhT{g}")
        nc.scalar.copy(uhT[:], pu[:])
        # iter-0 shortcut input: v0T[32j+nk, 1] = sum_s u_hatT
        v0T = uhT_pool.tile([128, 1], F32, tag="v0T")
        nc.vector.reduce_sum(v0T[:], uhT[:], axis=AX.X)
        # transpose back + compact: pnat [128 s_lo, (sh, j, nk)] fp16
        pnat = pnat_pool.tile([128, NSB * QUAD * NK], FP16, tag="pnat")
        for sh in range(NSB):
            nc.tensor.matmul(
                pnat[:, sh * QUAD * NK:(sh + 1) * QUAD * NK],
                uhT[:, sh * 128:(sh + 1) * 128],
                gat_h[:, 0:QUAD * NK],
                is_transpose=True,
                start=True,
                stop=True,
            )
        return pnat, v0T, uhT

    # ---------------- routing: one iteration of one group ----------------
    def route_iter(st, it):
        """Emit routing iteration `it` for group state `st`.

        Engines run their instruction queues IN ORDER, so a full group
        chain emitted contiguously blocks every later group's ready work
        behind its semaphore waits.  The main loop therefore calls this
        in a diagonal wavefront over (group, iter) so each engine queue
        interleaves independent groups' ops.
        """
        g, b_off, G = st["g"], st["b_off"], st["G"]
        uh_ap = st["uh_ap"]
        blog = st["blog"]
        v_src = st["v_src"]
        if True:
            cs = 1.0 / NCAP if it == 0 else 1.0
            t_src = st["uh"]
            if it > 0:
                # c = softmax_n(blog); t = c * u_hat  (blog lives in PSUM
                # f32, written by the b-update transpose-matmuls)
                expb = rt_pool.tile([128, NSB * G * NCAP], FP16, tag=f"expb{g}")
                nc.scalar.activation(expb[:], blog[:], AF.Exp)
                den = rt_pool.tile([128, NSB * G], F32, tag=f"den{g}")
                nc.vector.reduce_sum(
                    den[:],
                    expb[:].rearrange("p (sb n) -> p sb n", n=NCAP),
                    axis=AX.X,
                )
                rden = rt_pool.tile([128, NSB * G], F32, tag=f"rden{g}")
                nc.vector.reciprocal(rden[:], den[:])
                # flat 3D APs keep the DVE off its slow generic-AP path
                # (measured: the 5D broadcast forms ran ~40% slower)
                c = rt_pool.tile([128, NSB * G * NCAP], FP16, tag=f"c{g}")
                nc.vector.tensor_tensor(
                    c[:].rearrange("p (sb n) -> p sb n", n=NCAP),
                    expb[:].rearrange("p (sb n) -> p sb n", n=NCAP),
                    rden[:].unsqueeze(2).broadcast_to((128, NSB * G, NCAP)),
                    op=OP.mult,
                )
                t = rt_pool.tile([128, NSB * G * NK], FP16, tag=f"t{g}")
                nc.vector.tensor_tensor(
                    t[:].rearrange("p (sbn k) -> p sbn k", k=KDIM),
                    st["uh"][:].rearrange("p (sbn k) -> p sbn k", k=KDIM),
                    c[:].unsqueeze(2)
                    .broadcast_to((128, NSB * G * NCAP, KDIM)),
                    op=OP.mult,
                )
                # v[b,n,k] = sum_s t: 4 accumulating ones-matmuls over
                # the sh blocks.  v stays in PSUM — squash (ACT) and
                # vr/outs (DVE, one-psum-input rule satisfied) read it
                # there directly, so no copy-off and one fewer chain hop.
                pv = pv_pool.tile([1, 512], F32, tag="pv")
                t_ap = t[:].rearrange("p (sh rest) -> p sh rest", sh=NSB)
                for sh in range(NSB):
                    nc.tensor.matmul(
                        pv[0:1, 0:G * NK],
                        ones_col_b[:],
                        t_ap[:, sh],
                        start=(sh == 0),
                        stop=(sh == NSB - 1),
                    )
                v_src = pv
                st["v_src"] = pv
            # ---- squash side-chain: rnrm = exp(-0.5*ln(cs^2*|v|^2+eps))
            # Square/Ln/Exp all live in one ACT table set (no reloads).
            v_ap = v_src[0:1, 0:G * NK]
            sq = rt_pool.tile([1, G * NK], F32, tag=f"sq{g}")
            nc.scalar.activation(sq[:], v_ap, AF.Square)
            s2 = rt_pool.tile([1, G * NCAP], F32, tag=f"s2{g}")
            nc.vector.reduce_sum(
                s2[:], sq[:].rearrange("p (bn k) -> p bn k", k=KDIM), axis=AX.X
            )
            lnv = rt_pool.tile([1, G * NCAP], F32, tag=f"lnv{g}")
            nc.scalar.activation(lnv[:], s2[:], AF.Ln, bias=eps1[:], scale=cs * cs)
            rnrm = rt_pool.tile([1, G * NCAP], F32, tag=f"rnrm{g}")
            nc.scalar.activation(
                rnrm[:], lnv[:], AF.Exp,
                bias=ln_cs[:] if it == 0 else 0.0, scale=-0.5,
            )
            rnrm_b = rnrm[:].unsqueeze(2).broadcast_to((1, G * NCAP, KDIM))
            if it < ROUTINGS - 1:
                # vr = v * rnrm (iter-0 cs folded into rnrm via Exp
                # bias), written straight into 32-aligned blocks of a
                # [1,128] row (pads zeroed once per group).
                # b-update on the PE (was tmp+blog: 1.1us of DVE/iter):
                # vr row -> column via ones-matmul, M = mask*vr_col,
                # then blog_nat[s,(sh,b,n)] = sum_p uhT[p,s]*M[p,(b,n)]
                # per sh block in NORMAL matmul mode (general rhs).
                vr32 = rt_pool.tile([1, 128], FP16, tag=f"vr32{g}")
                if it == 0:
                    nc.gpsimd.memset(vr32[:], 0.0)
                nc.vector.tensor_tensor(
                    vr32[:].rearrange("p (j x) -> p j x", j=QUAD)
                    [:, :, 0:NK].rearrange("p j (n k) -> p j n k", n=NCAP),
                    v_ap.rearrange("p (j n k) -> p j n k", j=G, n=NCAP),
                    rnrm[:].rearrange("p (j n) -> p j n", j=G)
                    .unsqueeze(3).broadcast_to((1, G, NCAP, KDIM)),
                    op=OP.mult,
                )
                pvc = pvb_pool.tile([128, 1], F32, tag="pvc", bufs=1)
                nc.tensor.matmul(
                    pvc[:, 0:1], vr32[:], ones_col_b[0:1, 0:1],
                    start=True, stop=True,
                )
                em = rt_pool.tile([128, G * NCAP], FP16, tag=f"em{g}")
                nc.vector.tensor_tensor(
                    em[:],
                    mask[:, 0:G * NCAP],
                    pvc[:, 0:1].broadcast_to((128, G * NCAP)),
                    op=OP.mult,
                )
                blog = pvb_pool.tile(
                    [128, NSB * G * NCAP], F32, tag=f"pbl{g % 2}", bufs=1
                )
                for sh in range(NSB):
                    nc.tensor.matmul(
                        blog[:, sh * G * NCAP:(sh + 1) * G * NCAP],
                        st["uhT"][:, sh * 128:(sh + 1) * 128],
                        em[:],
                        start=True,
                        stop=True,
                    )
                st["blog"] = blog
            else:
                outs = rt_pool.tile([1, G * NK], F32, tag=f"outs{g}")
                nc.vector.tensor_tensor(
                    outs[:].rearrange("p (bn k) -> p bn k", k=KDIM),
                    v_ap.rearrange("p (bn k) -> p bn k", k=KDIM),
                    rnrm_b,
                    op=OP.mult,
                )
                nc.sync.dma_start(
                    out[0:1, b_off * NK:(b_off + G) * NK], outs[0:1, :]
                )

    # ---------------- main loop: diagonal wavefront ----------------
    # Slot s emits phase 1 of group s (one quad) plus routing iteration
    # (s - 1 - g) of every group g that has data — so every engine's
    # queue interleaves independent groups' ops.
    ngr = len(groups)
    states = []

    def phase1_group(g, G, b_off):
        assert G == QUAD
        uh = uh_pool.tile([128, NSB * G * NK], FP16, tag=f"uh{g}")
        pv0 = pv0_pool.tile([1, G * NK], F32, tag="pv0")
        pnat, v0T, uhT = phase1_post(g)
        # groups == quads, so pnat IS uh's layout: one plain ACT copy
        nc.scalar.copy(uh[:], pnat[:])
        # v0 row: pv0[0, (b, nk)] <- gather-transpose of v0T
        nc.tensor.matmul(
            pv0[0:1, 0:G * NK],
            v0T[:],
            gat_f[:, 0:G * NK],
            is_transpose=True,
            start=True,
            stop=True,
        )
        return {
            "g": g, "G": G, "b_off": b_off,
            "uh": uh, "uhT": uhT,
            "uh_ap": uh[:].rearrange(
                "p (sh b n k) -> p sh b n k", sh=NSB, b=G, n=NCAP
            ),
            "blog": None, "v_src": pv0,
        }

    # Readiness-interleaved schedule.  Groups 0-1 (first 8 batches:
    # exactly the 4 wait-free DMA issues each HWDGE queue allows) load
    # and route while the wire streams quads 2-3.  The second half's
    # DMA issues (which carry 4-deep semaphore-rotation waits) AND
    # quads 2-3's per-batch matmuls/posts are threaded BETWEEN routing
    # iterations at the points their inputs become ready — on in-order
    # engine queues nothing ever waits ahead of ready work, and groups
    # 2-3's routing starts as soon as their data lands instead of
    # queueing behind the whole first-half routing block.
    assert groups == [QUAD] * 4 and b_loc == 16
    for b in range(8):
        mm_batch(b)
        if b == 3:
            states.append(phase1_group(0, QUAD, 0))
    # R(0,0) before phase1_group(1): pv0 is a single rotating bank, so
    # each group's gather-transpose write must be emitted after the
    # previous group's iter-0 readers.
    route_iter(states[0], 0)
    states.append(phase1_group(1, QUAD, QUAD))
    issue_x(8)
    route_iter(states[1], 0)
    issue_x(9)
    route_iter(states[0], 1)
    issue_x(10)
    issue_x(11)
    mm_batch(8)
    mm_batch(9)
    route_iter(states[1], 1)
    mm_batch(10)
    mm_batch(11)
    route_iter(states[0], 2)
    issue_x(12)
    issue_x(13)
    states.append(phase1_group(2, QUAD, 8))
    route_iter(states[1], 2)
    mm_batch(12)
    mm_batch(13)
    route_iter(states[2], 0)
    route_iter(states[0], 3)
    issue_x(14)
    issue_x(15)
    mm_batch(14)
    mm_batch(15)
    route_iter(states[1], 3)
    states.append(phase1_group(3, QUAD, 12))
    # group 3's chain is the tail: start it before R(2,1) so the final
    # pair pipelines from the earliest possible point
    route_iter(states[3], 0)
    route_iter(states[2], 1)
    route_iter(states[2], 2)
    route_iter(states[3], 1)
    route_iter(states[2], 3)
    route_iter(states[3], 2)
    route_iter(states[3], 3)


def legalize_waits(nc):
    """This toolchain's walrus codegen accepts at most ONE sync wait per
    instruction ("Too many sync wait commands" otherwise) — and PE Matmult
    appears to take none safely. Hoist excess waits onto wait-only
    EventSemaphore instructions inserted just before, on the same engine
    (same pattern walrus already accepts for Tile's engine barriers)."""
    n = 0
    for fn in nc.m.functions:
        for blk in fn.blocks:
            new = []
            for inst in blk.instructions:
                si = inst.sync_info
                if si is not None and len(si.on_wait) > 0:
                    waits = list(si.on_wait)
                    keep = 0 if type(inst).__name__ == "InstMatmult" else 1
                    if len(waits) > keep:
                        for wt in waits[: len(waits) - keep]:
                            ev = mybir.InstEventSemaphore(
                                name=f"I-waitfix-{nc.next_id()}"
                            )
                            ev.engine = inst.engine
                            ev.sync_info = mybir.SyncInfo(on_wait=[wt], on_update=[])
                            new.append(ev)
                            n += 1
                        si.on_wait = waits[len(waits) - keep:]
                new.append(inst)
            blk.instructions = new
    return n


def build_caps_kernel(b_loc=16, groups=(4, 4, 4, 4)):
    nc = bass.Bass(trn_type="TRN2", debug=False, target_bir_lowering=False)
    x = nc.dram_tensor("x", [b_loc * 128, ND * S], FP16, kind="ExternalInput").ap()
    w = nc.dram_tensor("w", [128, ND * 32], FP16, kind="ExternalInput").ap()
    out = nc.dram_tensor("out", [1, b_loc * NK], F32, kind="ExternalOutput").ap()
    with tile.TileContext(nc) as tc:
        with ExitStack() as ctx:
            emit(ctx, tc, out, x, w, b_loc=b_loc, groups=groups)
    legalize_waits(nc)
    return nc


_KERNEL_CFG = dict(groups=(4, 4, 4, 4))


def _prep_inputs(x: np.ndarray, W: np.ndarray, b_loc: int):
    """Shard + host-side layout prep: fp16 cast and d-major transpose."""
    xb = x.astype(np.float16)
    wb = W.astype(np.float16)
    # W [D, 25] -> [d_lo(128), db(6)*32] with cols 25..31 zero-padded
    w_dev = np.zeros((128, ND, 32), dtype=np.float16)
    w_dev[:, :, :NK] = wb.reshape(ND, 128, NK).transpose(1, 0, 2)
    w_dev = np.ascontiguousarray(w_dev.reshape(128, ND * 32))
    in_maps = []
    for i in range(N_CORES):
        shard = xb[i * b_loc:(i + 1) * b_loc]  # [b_loc, S, D]
        # [b, s, db, d_lo] -> [b, d_lo, db, s]
        xt = np.ascontiguousarray(
            shard.reshape(b_loc, S, ND, 128).transpose(0, 3, 2, 1)
        ).reshape(b_loc * 128, ND * S)
        in_maps.append({"x": xt, "w": w_dev})
    return in_maps


def kernel(x: np.ndarray, W: np.ndarray) -> np.ndarray:
    from concourse.bass_utils import run_bass_kernel_spmd

    B, S_, D_ = x.shape
    assert (B, S_, D_) == (B_FULL, S, D)
    b_loc = B // N_CORES
    nc = build_caps_kernel(b_loc=b_loc, **_KERNEL_CFG)
    in_maps = _prep_inputs(x, W, b_loc)
    res = run_bass_kernel_spmd(nc, in_maps, core_ids=list(range(N_CORES)))
    outs = [res.results[i]["out"].reshape(b_loc, NCAP, KDIM) for i in range(N_CORES)]
    return np.concatenate(outs, axis=0).astype(np.float32)
